# revision 12
# baseline (speedup 1.0000x reference)
"""Trilinear interpolation (DeformationGrid) Bass kernel for 8 trn2 NeuronCores.

Data-parallel: coords/output sharded along the point axis across 8 cores;
theta replicated. Per core:
  1. Build a corner-pair table in HBM (fp16): row r=(i*127+j)*128+k holds
     theta[i..i+1, j..j+1, k, :] (12 halves = 24B). A point's 8 corners are
     rows r (z=k) and r+1 (z=k+1) — adjacent, so ONE 48B gather descriptor
     per point covers them. Build streams theta through SBUF with a
     cast-on-DMA fp32->fp16 load (SWDGE), an SBUF->SBUF partition-shift
     copy for the y+1 slab, and strided DVE copies.
  2. Stream points: compute cell ids + trilinear corner weights (DVE/ACT),
     gather per-point 48B blocks via ONE batched indirect DMA per tile
     (B indices per partition per instruction), then a strided fp16
     multiply + add-reduce forms the output.
Raw-bass implementation (manual semaphores) — Tile's attached waits exceed
the 1-wait limit of DMA pseudo-instructions for this dependency pattern.
"""
import sys

sys.path.insert(0, "/opt/trn_rl_repo")

import numpy as np

from concourse import bass, mybir
from concourse.bass import AP
from concourse.bass_utils import run_bass_kernel_spmd

N_CORES = 8
N_TOTAL = 4194304
NP = N_TOTAL // N_CORES        # 524288 points per core
G = 128
GD = G - 1                     # 127
ROW = 12                       # fp16 elems per table row (24B)
NROWS = GD * GD * G            # 2064512 table rows
ENT = 24                       # fp16 elems gathered per point (2 rows)
P = 128
B = 256                        # points per partition per stream tile
IW = 4                         # x-slices per build tile

F16 = mybir.dt.float16
F32 = mybir.dt.float32
I32 = mybir.dt.int32
OP = mybir.AluOpType
ACTF = mybir.ActivationFunctionType


def mkap(t_ap: AP, offset_elems: int, dims) -> AP:
    return AP(tensor=t_ap.tensor, offset=t_ap.offset + offset_elems, ap=list(dims))


def build_program(np_points: int = NP) -> bass.Bass:
    nc = bass.Bass()
    coords_in = nc.declare_dram_parameter("coords", [np_points, 3], F32, isOutput=False)
    theta_in = nc.declare_dram_parameter("theta", [G * G * G, 3], F32, isOutput=False)
    out_ext = nc.declare_dram_parameter("out", [np_points, 3], F32, isOutput=True)
    tbl = nc.dram_tensor("tbl", [NROWS, ROW], F16)

    T = np_points // (P * B)
    assert T * P * B == np_points
    TT = T + 1   # tile 0 is re-processed at the end (cold-start redo)
    bt = [(i0, min(IW, GD - i0)) for i0 in range(0, GD, IW)]
    NTB = len(bt)
    LDW = (IW + 1) * G * 3     # fp16 elems per partition per loaded slab
    OTW = IW * G * ROW         # fp16 elems per partition per staged out slab

    from contextlib import ExitStack
    with ExitStack() as _ctx:
        A16 = _ctx.enter_context(nc.sbuf_tensor([P, 2 * LDW], F16))
        S16 = _ctx.enter_context(nc.sbuf_tensor([P, 2 * LDW], F16))
        OT = _ctx.enter_context(nc.sbuf_tensor([P, 2 * OTW], F16))
        CO = _ctx.enter_context(nc.sbuf_tensor([P, 2 * B * 3], F32))
        XF = _ctx.enter_context(nc.sbuf_tensor([P, 2 * B * 3], F32))
        FR = _ctx.enter_context(nc.sbuf_tensor([P, 2 * B * 3], F32))
        I0 = _ctx.enter_context(nc.sbuf_tensor([P, 2 * B * 3], F32))
        T1 = _ctx.enter_context(nc.sbuf_tensor([P, 2 * B], F32))
        T2 = _ctx.enter_context(nc.sbuf_tensor([P, 2 * B], F32))
        IDX = _ctx.enter_context(nc.sbuf_tensor([P, 2 * B], I32))
        W2X = _ctx.enter_context(nc.sbuf_tensor([P, 2 * B * 2], F16))
        W2Y = _ctx.enter_context(nc.sbuf_tensor([P, 2 * B * 2], F16))
        W2Z = _ctx.enter_context(nc.sbuf_tensor([P, 2 * B * 2], F16))
        M4 = _ctx.enter_context(nc.sbuf_tensor([P, 2 * B * 4], F16))
        W8 = _ctx.enter_context(nc.sbuf_tensor([P, 2 * B * 8], F16))
        GT = _ctx.enter_context(nc.sbuf_tensor([P, 2 * B * ENT], F16))
        PR = _ctx.enter_context(nc.sbuf_tensor([P, 2 * B * ENT], F16))
        O3 = _ctx.enter_context(nc.sbuf_tensor([P, 2 * B * 3], F32))
        TG = _ctx.enter_context(nc.sbuf_tensor([P, 2 * B * 3], F32))
        dmaL = _ctx.enter_context(nc.semaphore("dmaL"))
        dmaS = _ctx.enter_context(nc.semaphore("dmaS"))
        dmaW = _ctx.enter_context(nc.semaphore("dmaW"))
        dmaC = _ctx.enter_context(nc.semaphore("dmaC"))
        dmaO = _ctx.enter_context(nc.semaphore("dmaO"))
        dmaG = [_ctx.enter_context(nc.semaphore(f"dmaG{i}")) for i in range(8)]
        vec_b = _ctx.enter_context(nc.semaphore("vec_b"))
        v_fr = _ctx.enter_context(nc.semaphore("v_fr"))
        v_idx = _ctx.enter_context(nc.semaphore("v_idx"))
        v_w8 = _ctx.enter_context(nc.semaphore("v_w8"))
        act_s = _ctx.enter_context(nc.semaphore("act_s"))
        v_xf = _ctx.enter_context(nc.semaphore("v_xf"))
        v_o3 = _ctx.enter_context(nc.semaphore("v_o3"))
        dmaWm = _ctx.enter_context(nc.semaphore("dmaWm"))
        block = _ctx.enter_context(nc.Block())

        @block.sync
        def _(sync):
            # stream loads + output writes (tile 0 redone as iteration T)
            for t in range(TT):
                ft = t if t < T else 0
                if t >= 2:
                    sync.wait_ge(v_xf, t - 1)
                sync.dma_start(
                    out=mkap(CO[:], (t % 2) * B * 3, [[2 * B * 3, P], [1, B * 3]]),
                    in_=mkap(coords_in[:], ft * P * B * 3, [[B * 3, P], [1, B * 3]]),
                ).then_inc(dmaC, 16)
                if t >= 1:
                    ftp = (t - 1) if (t - 1) < T else 0
                    sync.wait_ge(v_o3, t)
                    sync.dma_start(
                        out=mkap(out_ext[:], ftp * P * B * 3,
                                 [[B * 3, P], [1, B * 3]]),
                        in_=mkap(O3[:], ((t - 1) % 2) * B * 3,
                                 [[2 * B * 3, P], [1, B * 3]]),
                    ).then_inc(dmaO, 16)
            sync.wait_ge(v_o3, TT)
            sync.dma_start(
                out=mkap(out_ext[:], 0, [[B * 3, P], [1, B * 3]]),
                in_=mkap(O3[:], ((TT - 1) % 2) * B * 3, [[2 * B * 3, P], [1, B * 3]]),
            ).then_inc(dmaO, 16)
            sync.wait_ge(dmaO, 16 * TT)

        @block.scalar
        def _(scalar):
            # stream: w2[:, 0::2] = 1 - frac
            for t in range(TT):
                s3 = (t % 2) * B * 3
                s2 = (t % 2) * B * 2
                scalar.wait_ge(v_fr, t + 1)
                if t >= 2:
                    scalar.wait_ge(v_w8, t - 1)
                for c, W2 in ((0, W2X), (1, W2Y), (2, W2Z)):
                    nc.scalar.activation(
                        mkap(W2[:], s2, [[2 * B * 2, P], [2, B]]),
                        mkap(FR[:], s3 + c, [[2 * B * 3, P], [3, B]]),
                        ACTF.Copy, bias=1.0, scale=-1.0,
                    ).then_inc(act_s, 1)

        @block.vector
        def _(vector):
            # table build: scatter loaded slabs into corner-pair rows
            for bi, (i0, inn) in enumerate(bt):
                s = (bi % 2) * LDW
                so = (bi % 2) * OTW
                vector.wait_ge(dmaL, 16 * (bi + 1))
                vector.wait_ge(dmaS, 16 * (bi + 1))
                if bi >= 2:
                    vector.wait_ge(dmaW, 16 * (bi - 1))
                for di, dj in ((0, 0), (0, 1), (1, 0), (1, 1)):
                    srcbuf = (A16 if dj == 0 else S16)
                    nc.vector.tensor_copy(
                        mkap(OT[:], so + (di * 2 + dj) * 3,
                             [[2 * OTW, GD], [G * ROW, inn], [ROW, G], [1, 3]]),
                        mkap(srcbuf[:], s + di * (G * 3),
                             [[2 * LDW, GD], [G * 3, inn], [3, G], [1, 3]]),
                    ).then_inc(vec_b, 1)
            for t in range(TT):
                s3 = (t % 2) * B * 3
                s1 = (t % 2) * B
                s2 = (t % 2) * B * 2
                s4 = (t % 2) * B * 4
                s8 = (t % 2) * B * 8
                vector.wait_ge(dmaC, 16 * (t + 1))
                if t >= 2:
                    # IDX/GT slot reuse: gather of t-2 must be fully issued
                    vector.wait_ge(dmaG[(t - 2) % 8], 16 * B * ((t - 2) // 8 + 1))
                co = mkap(CO[:], s3, [[2 * B * 3, P], [1, B * 3]])
                xf = mkap(XF[:], s3, [[2 * B * 3, P], [1, B * 3]])
                fr = mkap(FR[:], s3, [[2 * B * 3, P], [1, B * 3]])
                i0v = mkap(I0[:], s3, [[2 * B * 3, P], [1, B * 3]])
                tg = mkap(TG[:], s3, [[2 * B * 3, P], [1, B * 3]])
                nc.vector.tensor_scalar(xf, co, float(GD), None, OP.mult).then_inc(v_xf, 1)
                # floor(xf): round via magic number, then subtract (rounded > xf)
                nc.vector.tensor_scalar(i0v, xf, 8388608.0, 8388608.0, OP.add, OP.subtract)
                nc.vector.tensor_tensor(tg, i0v, xf, OP.is_gt)
                nc.vector.tensor_tensor(i0v, i0v, tg, OP.subtract)
                nc.vector.tensor_tensor(fr, xf, i0v, OP.subtract).then_inc(v_fr, 1)
                i03 = [mkap(I0[:], s3 + c, [[2 * B * 3, P], [3, B]]) for c in range(3)]
                t1 = mkap(T1[:], s1, [[2 * B, P], [1, B]])
                t2 = mkap(T2[:], s1, [[2 * B, P], [1, B]])
                idx = mkap(IDX[:], s1, [[2 * B, P], [1, B]])
                nc.vector.scalar_tensor_tensor(t1, i03[0], float(GD), i03[1], OP.mult, OP.add)
                nc.vector.scalar_tensor_tensor(t2, t1, float(G), i03[2], OP.mult, OP.add)
                nc.vector.tensor_scalar(t2, t2, 0.0, float(NROWS - 2), OP.max, OP.min)
                nc.vector.tensor_copy(idx, t2).then_inc(v_idx, 1)
                # second halves of weight pairs (w)
                vector.wait_ge(act_s, 3 * (t + 1))
                for c, W2 in ((0, W2X), (1, W2Y), (2, W2Z)):
                    nc.vector.tensor_copy(
                        mkap(W2[:], s2 + 1, [[2 * B * 2, P], [2, B]]),
                        mkap(FR[:], s3 + c, [[2 * B * 3, P], [3, B]]))
                nc.vector.tensor_tensor(
                    mkap(M4[:], s4, [[2 * B * 4, P], [1, B * 4]]),
                    mkap(W2X[:], s2, [[2 * B * 2, P], [2, B], [1, 2], [0, 2]]),
                    mkap(W2Y[:], s2, [[2 * B * 2, P], [2, B], [0, 2], [1, 2]]),
                    OP.mult)
                # W8 order: z outer, (x,y) inner — matches 2-row gather layout
                nc.vector.tensor_tensor(
                    mkap(W8[:], s8, [[2 * B * 8, P], [1, B * 8]]),
                    mkap(M4[:], s4, [[2 * B * 4, P], [4, B], [0, 2], [1, 4]]),
                    mkap(W2Z[:], s2, [[2 * B * 2, P], [2, B], [1, 2], [0, 4]]),
                    OP.mult).then_inc(v_w8, 1)
                if t >= 1:
                    sEp = ((t - 1) % 2) * B * ENT
                    s8p = ((t - 1) % 2) * B * 8
                    s3p = ((t - 1) % 2) * B * 3
                    vector.wait_ge(dmaG[(t - 1) % 8], 16 * B * ((t - 1) // 8 + 1))
                    nc.vector.tensor_tensor(
                        mkap(PR[:], sEp, [[2 * B * ENT, P], [1, B * ENT]]),
                        mkap(GT[:], sEp, [[2 * B * ENT, P], [1, B * ENT]]),
                        mkap(W8[:], s8p, [[2 * B * 8, P], [8, B], [1, 8], [0, 3]]),
                        OP.mult)
                    nc.vector.tensor_reduce(
                        mkap(O3[:], s3p, [[2 * B * 3, P], [1, B * 3]]),
                        mkap(PR[:], sEp, [[2 * B * ENT, P], [ENT, B], [1, 3], [3, 8]]),
                        axis=mybir.AxisListType.X, op=OP.add).then_inc(v_o3, 1)
            sEp = ((TT - 1) % 2) * B * ENT
            s8p = ((TT - 1) % 2) * B * 8
            s3p = ((TT - 1) % 2) * B * 3
            vector.wait_ge(dmaG[(TT - 1) % 8], 16 * B * ((TT - 1) // 8 + 1))
            nc.vector.tensor_tensor(
                mkap(PR[:], sEp, [[2 * B * ENT, P], [1, B * ENT]]),
                mkap(GT[:], sEp, [[2 * B * ENT, P], [1, B * ENT]]),
                mkap(W8[:], s8p, [[2 * B * 8, P], [8, B], [1, 8], [0, 3]]),
                OP.mult)
            nc.vector.tensor_reduce(
                mkap(O3[:], s3p, [[2 * B * 3, P], [1, B * 3]]),
                mkap(PR[:], sEp, [[2 * B * ENT, P], [ENT, B], [1, 3], [3, 8]]),
                axis=mybir.AxisListType.X, op=OP.add).then_inc(v_o3, 1)

        @block.gpsimd
        def _(gpsimd):
            # build: loads (cast fp32->fp16), y+1 partition shifts, table
            # writes — all SWDGE (HWDGE writes observed to serialize on one
            # SDMA engine; SWDGE spreads by partition)
            for bi, (i0, inn) in enumerate(bt):
                nld = inn + 1
                s = (bi % 2) * LDW
                if bi >= 2:
                    gpsimd.wait_ge(vec_b, 4 * (bi - 1))
                gpsimd.dma_start(
                    out=mkap(A16[:], s, [[2 * LDW, P], [G * 3, nld], [1, G * 3]]),
                    in_=mkap(theta_in[:], i0 * G * G * 3,
                             [[G * 3, P], [G * G * 3, nld], [1, G * 3]]),
                ).then_inc(dmaL, 16)
                gpsimd.wait_ge(dmaL, 16 * (bi + 1))
                gpsimd.dma_start(
                    out=mkap(S16[:], s, [[2 * LDW, P - 1], [1, nld * G * 3]]),
                    in_=mkap(A16[:], s + 2 * LDW, [[2 * LDW, P - 1], [1, nld * G * 3]]),
                ).then_inc(dmaS, 16)
                if bi >= 1:
                    p0, pn = bt[bi - 1]
                    gpsimd.wait_ge(vec_b, 4 * bi)
                    so = ((bi - 1) % 2) * OTW
                    gpsimd.dma_start(
                        out=mkap(tbl[:], p0 * GD * G * ROW,
                                 [[G * ROW, GD], [GD * G * ROW, pn], [1, G * ROW]]),
                        in_=mkap(OT[:], so,
                                 [[2 * OTW, GD], [G * ROW, pn], [1, G * ROW]]),
                    ).then_inc(dmaW, 16)
            p0, pn = bt[NTB - 1]
            gpsimd.wait_ge(vec_b, 4 * NTB)
            so = ((NTB - 1) % 2) * OTW
            gpsimd.dma_start(
                out=mkap(tbl[:], p0 * GD * G * ROW,
                         [[G * ROW, GD], [GD * G * ROW, pn], [1, G * ROW]]),
                in_=mkap(OT[:], so,
                         [[2 * OTW, GD], [G * ROW, pn], [1, G * ROW]]),
            ).then_inc(dmaW, 16)
            gpsimd.wait_ge(dmaW, 16 * NTB)
            # SWDGE warm-up: duplicate tile-0 gathers; the real pass below
            # overwrites in ring-FIFO order (cold rings corrupt the first
            # batch on engine 0 otherwise)
            gpsimd.wait_ge(v_idx, 1)
            for k in range(B):
                gpsimd.indirect_dma_start(
                    out=mkap(GT[:], k * ENT, [[2 * B * ENT, P], [1, ENT]]),
                    out_offset=None,
                    in_=tbl[:].rearrange("(a b) e -> a b e", a=GD),
                    in_offset=bass.IndirectOffsetOnAxis(
                        ap=mkap(IDX[:], k, [[2 * B, P], [1, 1]]), axis=1),
                ).then_inc(dmaWm, 16)
            gpsimd.wait_ge(dmaWm, 16 * B)
            for t in range(TT):
                sE = (t % 2) * B * ENT
                s1 = (t % 2) * B
                gpsimd.wait_ge(v_idx, t + 1)
                if t >= 2:
                    gpsimd.wait_ge(v_o3, t - 1)
                for k in range(B):
                    gpsimd.indirect_dma_start(
                        out=mkap(GT[:], sE + k * ENT, [[2 * B * ENT, P], [1, ENT]]),
                        out_offset=None,
                        in_=tbl[:].rearrange("(a b) e -> a b e", a=GD),
                        in_offset=bass.IndirectOffsetOnAxis(
                            ap=mkap(IDX[:], s1 + k, [[2 * B, P], [1, 1]]), axis=1),
                    ).then_inc(dmaG[t % 8], 16)
    return nc


_CACHED = {}


def _get_program():
    if "nc" not in _CACHED:
        _CACHED["nc"] = build_program()
    return _CACHED["nc"]


def kernel(coords: np.ndarray, theta: np.ndarray) -> np.ndarray:
    coords = np.ascontiguousarray(coords, dtype=np.float32)
    theta = np.ascontiguousarray(theta, dtype=np.float32).reshape(G * G * G, 3)
    nc = _get_program()
    shards = coords.reshape(N_CORES, NP, 3)
    in_maps = [{"coords": shards[i], "theta": theta} for i in range(N_CORES)]
    res = run_bass_kernel_spmd(nc, in_maps, list(range(N_CORES)))
    out = np.concatenate([res.results[i]["out"] for i in range(N_CORES)], axis=0)
    return out.reshape(N_TOTAL, 3)
